# revision 27
# baseline (speedup 1.0000x reference)
"""Trainium2 Bass kernel for a dense multi-head self-attention block.

Computation (matches torch/diffusers Attention with upcast softmax):
    q/k/v = hs @ W.T + b ; per-head scaled QK^T ; softmax ; PV ; out proj.
Shapes: hs [2, 2048, 1024], 16 heads x 64 dim, fp32 in/out.

Sharding: batch*head parallel over 8 cores. Core c owns heads {2c, 2c+1}
(feature slice c*128:(c+1)*128 of E) for both batches. The host pre-packs
hidden_states and weights into partition-major fp16 layouts; the device
never transposes activations. Per core:
  - Q^T/K^T/V^T projections for its 128 features over all 4096 tokens
    (fp16 operands, fp32 PSUM accumulation); V^T re-tiled to
    [tokens, features] via PE transposes with an all-ones column so the
    PV matmul also accumulates the softmax denominator (row 64).
  - attention in scores^T layout (K @ Q^T). The two heads' QK matmuls
    are K=64 row-tiles at partition bases 0/64 -> the PE packs the pair
    concurrently (measured 3ns pair stagger, ~259ns per packed pair vs
    ~518ns serial: 2x QK throughput).
  - phase 2 is a flat 128-slot pipeline over (qblock512 x ktile): slot s
    emits exp(s) (one head on ScalarE exact exp, one on DVE as a
    Schraudolph fp16 bit-trick: i16 = s*1024*log2(e)*SCALE + c), QK(s+1)
    as a packed pair, PV(s-1). Score PSUM tiles are 1-bank [128,512]
    with a 4-deep ring (2 slots of lookahead) so QK rarely waits on the
    exp engines draining PSUM.
  - softmax normalization (denominator reciprocal via a [128,8]
    pack-dance + DMA broadcast, all on the otherwise-idle GpSimd DMA
    queue) and the partial out-projection for q-block N are spread over
    q-block N+1's 16 slots (out-proj as 2-bank eb-pairs with one wide
    PSUM-evict copy), so the 8MB fp16 partial-output DMA streams during
    attention instead of forming a tail.
  - the LAST q-block (tokens 3584:4096) skips the on-device
    normalization chain entirely: per-head unnormalized K=64 out-proj
    partials (out7) + raw denominators (den7) are emitted straight off
    the PV copies, and the host folds in 1/d. This removes an ~11us
    latency-bound DMA round-trip chain from the kernel tail.
  - PSUM: 8 banks = score ring 4 + PV accumulators 2 + out-proj pair 2.
  - the host sums the 8 partial outputs (+ the qb7 fold) + o_b.

Timing on this 8-core axon pod: ~226-233us HW exec (was ~271-277us for
the previous build on the same pod), rel err 6.4e-3. Attention runs
warm (2.4GHz HAM state) and PE-dense: ~92% tensor-engine busy in the
attention window.
"""

import numpy as np

import concourse.bass as bass
import concourse.mybir as mybir
import concourse.tile as tile
from concourse import bacc
from concourse.bass_utils import run_bass_kernel_spmd

B, S, E = 2, 2048, 1024
H, D = 16, 64
SCALE = D ** -0.5
NCORE = 8
T = B * S              # 4096 tokens
FPC = 128              # features per core (2 heads x 64)
HPC = 2                # heads per core

F32 = mybir.dt.float32
F16 = mybir.dt.float16
I16 = mybir.dt.int16
EXP = mybir.ActivationFunctionType.Exp
MULT = mybir.AluOpType.mult
ADD = mybir.AluOpType.add

# Schraudolph exp2 constants (fp16 bit trick), scale folded in:
#   i16 = s * SCALE * 1024/ln(2) + 15360 + c
SCH_A = SCALE * 1024.0 / float(np.log(2.0))
SCH_C = 15360.0 - 45.0

# set by test harness to profile; results stashed in LAST_RESULT
TRACE = False
DEBUG = False
LAST_RESULT = None
_CACHE = {}


def _build(ctx, tc, io):
    nc = tc.nc
    hs_p, wq_p, wk_p, wv_p, ow_t, out_p = (
        io["hs_p"], io["wq_p"], io["wk_p"], io["wv_p"], io["ow_t"], io["out_p"],
    )

    # ---------------- pools ----------------
    consts = ctx.enter_context(tc.tile_pool(name="consts", bufs=1))
    persist = ctx.enter_context(tc.tile_pool(name="persist", bufs=1))
    hst_pool = ctx.enter_context(tc.tile_pool(name="hst", bufs=4))
    vt_pool = ctx.enter_context(tc.tile_pool(name="vt", bufs=3))
    pt_pool = ctx.enter_context(tc.tile_pool(name="pt", bufs=6))
    bc_pool = ctx.enter_context(tc.tile_pool(name="bcs", bufs=4))
    rc_pool = ctx.enter_context(tc.tile_pool(name="rc", bufs=2))
    at_pool = ctx.enter_context(tc.tile_pool(name="at", bufs=2))
    out_pool = ctx.enter_context(tc.tile_pool(name="outs", bufs=6))
    dr_pool = ctx.enter_context(tc.tile_pool(name="drb", bufs=2, space="DRAM"))
    # PSUM: 8 banks = sc ring 4 x [128,512]f32 + pv 2 + op 1x[128,1024] (2)
    psum = ctx.enter_context(tc.tile_pool(name="psum", bufs=4, space="PSUM"))
    sc_tile = lambda nm: psum.tile([128, 512], F32, tag="sc", bufs=4, name=nm)
    pv_tile = lambda nm: psum.tile([65, 512], F32, tag="pv", bufs=2, name=nm)
    tp_tile = lambda nm: psum.tile([128, 128], F16, tag="pv", bufs=2, name=nm)
    op_tile = lambda nm: psum.tile([128, 1024], F32, tag="op", bufs=1, name=nm)

    # ---------------- constants / weights ----------------
    wq_sb = consts.tile([128, 8, 128], F16, tag="wq")
    wk_sb = consts.tile([128, 8, 128], F16, tag="wk")
    wv_sb = consts.tile([128, 8, 128], F16, tag="wv")
    ow_sb = consts.tile([128, 1024], F16, tag="ow")
    bias_sb = consts.tile([128, 3], F32, tag="bias")
    qb_sb, kb_sb, vb_sb = bias_sb[:, 0:1], bias_sb[:, 1:2], bias_sb[:, 2:3]
    cpack = consts.tile([128, 144], F16, tag="cpack")
    ident = cpack[:, 0:128]

    # head-1 rows of the out-proj weight re-staged at partition base 0 for
    # the final q-block's per-head out-projection (host-side normalization)
    ow2_sb = consts.tile([64, 1024], F16, tag="ow2")

    # wq + bias first on the GpSimd queue (first matmuls need only these +
    # hst0's first half); Sync carries only the hs stream + outputs, so the
    # hs desc-gen is never stuck behind weight DMAs; GpSimd has no table
    # load at startup, so it carries the weights.
    nc.gpsimd.dma_start(wq_sb[:, 0:4], wq_p[:, 0:4])
    nc.gpsimd.dma_start(bias_sb[:], io["bias3"][:])
    nc.gpsimd.dma_start(wq_sb[:, 4:8], wq_p[:, 4:8])

    # persistent activations: feature dim (128 = 2 heads x 64) on partitions
    qt_sb = persist.tile([128, T], F16, tag="qt")      # Q^T
    kt_sb = persist.tile([128, T], F16, tag="kt")      # K^T
    v_bh = [
        [
            persist.tile([128, 16, 65], F16, tag=f"v{b}{h}", name=f"v{b}{h}")
            for h in range(2)
        ]
        for b in range(B)
    ]

    # ---------------- phase 1: QKV projections ----------------
    # hst DMAs are prefetched 4 token-blocks deep (= the hst ring depth) so
    # the Sync queue desc-gens them back-to-back from t~7us; without this,
    # tb0's second half's DESCRIPTOR is only generated at ~15us and the PE
    # sits idle (and HAM-cold) for ~6us early in phase 1.
    hst_tiles = []

    def prefetch_hst(ptb):
        ha = hst_pool.tile([128, 4, 512], F16, tag="hsta", name="hst_a")
        nc.sync.dma_start(ha[:, 0:2], hs_p[:, ptb, 0:2])
        nc.sync.dma_start(ha[:, 2:4], hs_p[:, ptb, 2:4])
        hb = hst_pool.tile([128, 4, 512], F16, tag="hstb", name="hst_b")
        nc.sync.dma_start(hb[:], hs_p[:, ptb, 4:8])
        hst_tiles.append((ha, hb))

    for ptb in range(4):
        prefetch_hst(ptb)
    for tb in range(8):                      # 512-token blocks over B*S
        hst_a, hst_b = hst_tiles[tb]
        if tb == 0:
            nc.gpsimd.dma_start(wk_sb[:], wk_p[:])
            nc.gpsimd.dma_start(wv_sb[:], wv_p[:])
            nc.gpsimd.dma_start(cpack[:], io["cpack"][:])
            nc.gpsimd.dma_start(ow_sb[:], ow_t[:])
            nc.gpsimd.dma_start(ow2_sb[:], ow_t[64:128, :])
        hs_et = lambda et: (hst_a if et < 4 else hst_b)[:, et % 4, :]
        for w_sb, b_sb, dest in ((wq_sb, qb_sb, qt_sb), (wk_sb, kb_sb, kt_sb)):
            ps = sc_tile("ps")
            for et in range(8):
                nc.tensor.matmul(
                    ps[:], w_sb[:, et, :], hs_et(et),
                    start=(et == 0), stop=(et == 7),
                )
            nc.vector.tensor_scalar_add(
                dest[:, tb * 512:(tb + 1) * 512], ps[:], b_sb[:]
            )
        # V^T then transpose into [tokens, features] tiles; V rides the
        # (phase-2-only) op tile so Q/K keep the full sc ring to themselves
        vps = op_tile("vps")[:, 0:512]
        for et in range(8):
            nc.tensor.matmul(
                vps[:], wv_sb[:, et, :], hs_et(et),
                start=(et == 0), stop=(et == 7),
            )
        vtt = vt_pool.tile([128, 512], F16, tag="vtt")
        nc.vector.tensor_scalar_add(vtt[:], vps[:], vb_sb[:])
        b = tb // 4
        for j in range(4):
            ktl = (tb % 4) * 4 + j           # k-tile index within batch
            tps = tp_tile("tps")
            nc.tensor.transpose(tps[:], vtt[:, j * 128:(j + 1) * 128], ident[:])
            nc.vector.tensor_copy(v_bh[b][0][:, ktl, 0:64], tps[:, 0:64])
            nc.vector.tensor_copy(v_bh[b][1][:, ktl, 0:64], tps[:, 64:128])
        # prefetch tb+4's hst AFTER this tb's reads are emitted, so the
        # ring-buffer write-after-read ordering is explicit
        if tb + 4 < 8:
            prefetch_hst(tb + 4)

    # ones column rides along in PV to accumulate the softmax denominator
    for b in range(B):
        for h in range(2):
            nc.vector.tensor_copy(
                v_bh[b][h][:, :, 64:65],
                cpack[:, 128:144].rearrange("p (a o) -> p a o", o=1),
            )

    # ---------------- phase 2: attention (+ streamed out-projection) ----
    # slots: 8 q-blocks of 512 tokens x 16 k-tiles of 128 tokens
    SLOTS = [(b, q4, kt) for b in range(B) for q4 in range(4)
             for kt in range(16)]

    def emit_qk(slot):
        b, q4, kt = slot
        qoff = b * S + q4 * 512
        koff = b * S + kt * 128
        sc = []
        for h in range(2):
            p0 = h * 64
            sch = sc_tile(f"sc{h}")
            # K=64 row-tiles at bases 0/64: the pair packs on the PE
            nc.tensor.matmul(
                sch[:], kt_sb[p0:p0 + 64, koff:koff + 128],
                qt_sb[p0:p0 + 64, qoff:qoff + 512],
                start=True, stop=True,
            )
            sc.append(sch)
        return sc

    def emit_exp(slot, sc):
        b, q4, kt = slot
        pt = []
        for h in range(2):
            pth = pt_pool.tile([128, 512], F16, tag="pt")
            if (kt + h) % 2 == 0:
                nc.scalar.activation(pth[:], sc[h][:], EXP, scale=SCALE)
            else:
                nc.vector.tensor_scalar(
                    pth[:].bitcast(I16), sc[h][:], SCH_A, SCH_C, MULT, ADD,
                )
            pt.append(pth)
        return pt

    def emit_pv(slot, pt, pv):
        b, q4, kt = slot
        for h in range(2):
            nc.tensor.matmul(
                pv[h][:], v_bh[b][h][:, kt, :], pt[h][:],
                start=(kt == 0), stop=(kt == 15),
            )

    # deferred per-qblock epilogue: normalization + out-projection,
    # dripped one small op per slot of the NEXT q-block.
    def arm_epilogue(qb_idx, pv):
        st = {"qb": qb_idx, "pv": pv}
        return st

    def emit_epilogue(st, kt):
        qb_idx = st["qb"]
        b, q4 = qb_idx // 4, qb_idx % 4
        qoff = b * S + q4 * 512
        if kt == 0:
            # copy PV out of PSUM (frees accumulator banks), split engines
            pvs = rc_pool.tile([65, 1024], F32, tag="pvs", name="pvs")
            nc.vector.tensor_copy(pvs[:, 0:512], st["pv"][0][:])
            nc.scalar.copy(pvs[:, 512:1024], st["pv"][1][:])
            st["pvs"] = pvs
        elif kt == 1:
            # denominators (row 64) -> DRAM -> [128,8] pack. The whole norm
            # chain (DMAs + muls) rides the idle GpSimd queue so its latency
            # never head-of-line-blocks the Sync queue or the exp engines.
            den_dr = dr_pool.tile([2, 512], F32, tag="den_dr", name="den_dr")
            nc.gpsimd.dma_start(
                den_dr.rearrange("a n -> (a n)"), st["pvs"][64:65, :]
            )
            dpack = rc_pool.tile([128, 8], F32, tag="rcp", name="dpack")
            nc.gpsimd.dma_start(
                dpack[:],
                den_dr.rearrange("a n -> (a n)").rearrange("(p i) -> p i", p=128),
            )
            st["dpack"] = dpack
        elif kt == 2:
            rpack = rc_pool.tile([128, 8], F32, tag="rcp", name="rpack")
            with nc.allow_low_precision(reason="softmax denom reciprocal"):
                nc.vector.reciprocal(rpack[:], st["dpack"][:])
            rcp_dr = dr_pool.tile([2, 512], F32, tag="rcp_dr", name="rcp_dr")
            nc.gpsimd.dma_start(
                rcp_dr.rearrange("a n -> (a n)").rearrange("(p i) -> p i", p=128),
                rpack[:],
            )
            st["rcp_dr"] = rcp_dr
        elif kt == 3:
            st["bc"] = []
            for h in range(2):
                bch = bc_pool.tile([64, 512], F32, tag="bcs", name=f"bc{h}")
                nc.gpsimd.dma_start(
                    bch[:], st["rcp_dr"][h:h + 1, :].broadcast_to([64, 512])
                )
                st["bc"].append(bch)
        elif kt == 5:
            at = at_pool.tile([128, 512], F16, tag="at", name="at")
            nc.vector.tensor_mul(
                at[0:64, :], st["pvs"][0:64, 0:512], st["bc"][0][:]
            )
            st["at"] = at
        elif kt == 6:
            a1 = vt_pool.tile([64, 512], F16, tag="a1", name="a1")
            nc.vector.tensor_mul(a1[:], st["pvs"][0:64, 512:1024], st["bc"][1][:])
            # head 1 lives on partitions 64:128 -- shift via SBUF->SBUF DMA
            nc.gpsimd.dma_start(st["at"][64:128, :], a1[:])
        elif kt in (8, 10, 12, 14):
            # out-proj: both eb halves of token-chunk c into one 2-bank op
            # tile, evicted by a single wide copy (halves the copy overhead
            # and the number of ops delaying the exp engines)
            c = (kt - 8) // 2
            if c == 0:
                st["ot"] = [None, None]
            if c % 2 == 0:
                st["ot"][c // 2] = out_pool.tile(
                    [128, 2, 1024], F16, tag="outs", name="ot"
                )
            ops = op_tile("ops")
            for eb in range(2):
                nc.tensor.matmul(
                    ops[:, eb * 512:(eb + 1) * 512],
                    st["at"][:, c * 128:(c + 1) * 128],
                    ow_sb[:, eb * 512:(eb + 1) * 512],
                    start=True, stop=True,
                )
            dst = st["ot"][c // 2][:, c % 2, :]
            if c % 2 == 0:
                nc.vector.tensor_copy(dst, ops[:])
            else:
                nc.scalar.copy(dst, ops[:])
        elif kt == 11 or kt == 15:
            half = (kt - 11) // 4
            t0 = qoff + half * 256
            # qb6's second half lands at tail-time: route it via the (then
            # idle) Scalar queue so the final wire drains on two DMA rings
            eng = nc.scalar if (qb_idx == 6 and kt == 15) else nc.sync
            eng.dma_start(
                out_p[t0:t0 + 256, :].rearrange("(a p) n -> p a n", p=128),
                st["ot"][half][:],
            )

    sc_next = emit_qk(SLOTS[0])
    pt_prev = prev = pv_cur = None
    epi = None
    for s, slot in enumerate(SLOTS):
        b, q4, kt = slot
        sc = sc_next
        pt = emit_exp(slot, sc)
        if s + 1 < len(SLOTS):
            sc_next = emit_qk(SLOTS[s + 1])
        if prev is not None:
            pb, pq4, pkt = prev
            if pkt == 0:
                pv_cur = [pv_tile(f"pv{h}") for h in range(2)]
            emit_pv(prev, pt_prev, pv_cur)
            if pkt == 15:
                epi = arm_epilogue(pb * 4 + pq4, pv_cur)
        if epi is not None:
            emit_epilogue(epi, kt)
        pt_prev, prev = pt, slot
    # ---- tail: qb7 (tokens 3584:4096) with HOST-side normalization ----
    # Skips the ~11us denominator-reciprocal DMA chain: emit per-head
    # UNNORMALIZED out-proj partials (K=64 contractions straight off the
    # PV copies) + the raw denominators; the host folds in 1/d. Out-proj
    # pairs ride the freed sc ring so up to 2 pairs are in flight.
    emit_pv(prev, pt_prev, pv_cur)
    pvs = rc_pool.tile([65, 1024], F16, tag="pvs7", name="pvs7")
    nc.vector.tensor_copy(pvs[:, 0:512], pv_cur[0][:])
    nc.scalar.copy(pvs[:, 512:1024], pv_cur[1][:])
    nc.gpsimd.dma_start(io["den7"].rearrange("a n -> (a n)"), pvs[64:65, :])
    ot7 = {}
    for c in range(4):
        for h in range(2):
            if c % 2 == 0:
                ot7[(h, c // 2)] = out_pool.tile(
                    [128, 2, 1024], F16, tag="outs", name="ot7"
                )
            oww = ow_sb if h == 0 else ow2_sb
            for eb in range(2):
                ops = sc_tile("ops7")
                nc.tensor.matmul(
                    ops[:], pvs[0:64, h * 512 + c * 128:h * 512 + (c + 1) * 128],
                    oww[0:64, eb * 512:(eb + 1) * 512],
                    start=True, stop=True,
                )
                dst = ot7[(h, c // 2)][:, c % 2, eb * 512:(eb + 1) * 512]
                if (c * 2 + eb) % 2 == 0:
                    nc.vector.tensor_copy(dst, ops[:])
                else:
                    nc.scalar.copy(dst, ops[:])
        if c % 2 == 1:
            for h in range(2):
                ch = c // 2
                eng = nc.sync if h == 0 else nc.scalar
                eng.dma_start(
                    io["out7"][h, ch * 256:(ch + 1) * 256, :]
                    .rearrange("(a p) n -> p a n", p=128),
                    ot7[(h, ch)][:],
                )


def _get_program():
    if "nc" in _CACHE:
        return _CACHE["nc"]
    from contextlib import ExitStack

    nc = bacc.Bacc("TRN2", target_bir_lowering=False, debug=False,
                   num_devices=NCORE)
    io = {
        "hs_p": nc.dram_tensor("hs_p", [128, 8, 8, 512], F16, kind="ExternalInput").ap(),
        "wq_p": nc.dram_tensor("wq_p", [128, 8, 128], F16, kind="ExternalInput").ap(),
        "wk_p": nc.dram_tensor("wk_p", [128, 8, 128], F16, kind="ExternalInput").ap(),
        "wv_p": nc.dram_tensor("wv_p", [128, 8, 128], F16, kind="ExternalInput").ap(),
        "ow_t": nc.dram_tensor("ow_t", [FPC, E], F16, kind="ExternalInput").ap(),
        "bias3": nc.dram_tensor("bias3", [FPC, 3], F32, kind="ExternalInput").ap(),
        "cpack": nc.dram_tensor("cpack", [128, 144], F16, kind="ExternalInput").ap(),
        "out_p": nc.dram_tensor("out_p", [T, E], F16, kind="ExternalOutput").ap(),
        "out7": nc.dram_tensor("out7", [2, 512, E], F16, kind="ExternalOutput").ap(),
        "den7": nc.dram_tensor("den7", [2, 512], F16, kind="ExternalOutput").ap(),
    }
    with tile.TileContext(nc) as tc:
        with ExitStack() as ctx:
            _build(ctx, tc, io)
    nc.compile()
    _CACHE["nc"] = nc
    return nc


def kernel(hidden_states, q_w, q_b, k_w, k_b, v_w, v_b, o_w, o_b):
    global LAST_RESULT
    nc = _get_program()

    f32c = lambda a: np.ascontiguousarray(a, dtype=np.float32)
    f16c = lambda a: np.ascontiguousarray(a, dtype=np.float16)
    # hs_p[p, tb, et, n] = hs[token tb*512+n, feature et*128+p]
    hs_t = np.asarray(hidden_states, dtype=np.float32).reshape(T, E).T
    hs_pm = f16c(hs_t.reshape(8, 128, 8, 512).transpose(1, 2, 0, 3))
    wp = lambda w, sl: f16c(
        np.asarray(w)[sl, :].T.reshape(8, 128, FPC).transpose(1, 0, 2)
    )
    in_maps = []
    for c in range(NCORE):
        sl = slice(c * FPC, (c + 1) * FPC)
        in_maps.append({
            "hs_p": hs_pm,
            "wq_p": wp(q_w, sl),
            "wk_p": wp(k_w, sl),
            "wv_p": wp(v_w, sl),
            "ow_t": f16c(np.asarray(o_w)[:, sl].T),
            "bias3": f32c(np.stack([np.asarray(q_b)[sl], np.asarray(k_b)[sl],
                                     np.asarray(v_b)[sl]], axis=1)),
            "cpack": f16c(np.concatenate([np.eye(128, dtype=np.float16),
                                          np.ones((128, 16), np.float16)], axis=1)),
        })

    res = run_bass_kernel_spmd(nc, in_maps, list(range(NCORE)), trace=TRACE)
    LAST_RESULT = res
    out = res.results[0]["out_p"].astype(np.float64)
    for c in range(1, NCORE):
        out += res.results[c]["out_p"]
    # tokens 3584:4096 come as per-head unnormalized partials + denominators
    last = np.zeros((512, E), dtype=np.float64)
    for c in range(NCORE):
        o7 = res.results[c]["out7"].astype(np.float64)
        d7 = res.results[c]["den7"].astype(np.float64)
        last += o7[0] / d7[0][:, None] + o7[1] / d7[1][:, None]
    out[3584:4096] = last
    out += np.asarray(o_b, dtype=np.float64)
    return out.reshape(B, S, E).astype(np.float32)
